# revision 8
# baseline (speedup 1.0000x reference)
"""Self-attention (SAGAN-style) Trainium2 kernel.

Reference computation (per batch sample):
    theta = w_theta @ x            # [32, 4096]
    phi   = pool2x2(w_phi @ x)     # [32, 1024]
    g     = pool2x2(w_g @ x)       # [128, 1024]
    beta  = softmax(theta.T @ phi, axis=-1)   # [4096, 1024]
    attn  = g @ beta.T             # [128, 4096]
    out   = gamma * (w_o @ attn) + x

Sharding: data-parallel over batch; B=16 over 8 cores -> 2 samples/core.

Kernel layout strategy (per core, per sample):
  - projections: out[ch, n] = w.T-chunks (lhsT, fp32r) @ x-chunks, accumulated
    over the two 128-channel chunks of C=256.
  - scoresT in [m, n] layout (m on partitions, 8 chunks of 128): one K=32
    matmul per (mc, n-chunk); exp via ScalarE directly out of PSUM with bf16
    output (logits are O(+-40) so exp without max-subtraction is safe in f32
    PSUM -> bf16).
  - attn[c, n] = sum_mc gT[mc].T @ expT[mc]; gT built by PE transposes of the
    pooled g. The softmax denominator rides the same rhs stream through an
    all-ones stationary operand, which also broadcasts the row-sum to all 128
    partitions for free.
  - normalize with reciprocal_approx_fast + one scalar_tensor_tensor.
  - o = (gamma*w_o).T-chunks @ attn (bf16); residual added during PSUM
    evacuation with scalar_tensor_tensor; DMA straight out.
"""

import numpy as np

import concourse.bacc as bacc
import concourse.mybir as mybir
from concourse import tile
from concourse.bass_utils import run_bass_kernel_spmd

F32 = mybir.dt.float32
F32R = mybir.dt.float32r
BF16 = mybir.dt.bfloat16

B, C, H, W = 16, 256, 64, 64
N = H * W            # 4096
M = N // 4           # 1024
C8 = C // 8          # 32
C2 = C // 2          # 128
NCORES = 8
BPC = B // NCORES    # 2 samples per core
NCH = 512            # n-chunk width for matmul streaming
NNCH = N // NCH      # 8
MC = M // 128        # 8 m-chunks


def build_kernel():
    nc = bacc.Bacc("TRN2", target_bir_lowering=False, debug=False)

    x_d = nc.declare_dram_parameter("x", [BPC, C, N], F32R, isOutput=False)
    # weight chunks pre-transposed on host: [cc][128 chans][out-dim]
    wth_d = nc.declare_dram_parameter("wth", [2, 128, C8], F32R, isOutput=False)
    wph_d = nc.declare_dram_parameter("wph", [2, 128, C8], F32R, isOutput=False)
    wg_d = nc.declare_dram_parameter("wg", [2, 128, C2], F32R, isOutput=False)
    wo_d = nc.declare_dram_parameter("wo", [C2, C], F32, isOutput=False)  # (gamma*w_o).T
    id_d = nc.declare_dram_parameter("ident", [128, 128], F32, isOutput=False)
    out_d = nc.declare_dram_parameter("out", [BPC, C, N], F32, isOutput=True)

    with tile.TileContext(nc) as tc:
        with (
            tc.tile_pool(name="const", bufs=1) as constp,
            tc.tile_pool(name="xin", bufs=2) as xp,
            tc.tile_pool(name="proj", bufs=1) as projp,
            tc.tile_pool(name="exp", bufs=1) as expp,
            tc.tile_pool(name="gt", bufs=1) as gtp,
            tc.tile_pool(name="small", bufs=3) as smallp,
            tc.tile_pool(name="outs", bufs=4) as outp,
            tc.tile_pool(name="ps_big", bufs=2, space="PSUM") as psb,
            tc.tile_pool(name="ps_small", bufs=4, space="PSUM") as pss,
            # psb: one tag "big" [128,1024] x2 slots = 4 banks
            # pss: one tag "small" [128,512] x4 slots = 4 banks
        ):
            # ---- constants / weights (loaded once) ----
            wth, wph, wg = [], [], []
            for cc in range(2):
                t = constp.tile([128, C8], F32R, tag=f"wth{cc}")
                nc.sync.dma_start(t[:], wth_d[cc])
                wth.append(t)
                t = constp.tile([128, C8], F32R, tag=f"wph{cc}")
                nc.sync.dma_start(t[:], wph_d[cc])
                wph.append(t)
                t = constp.tile([128, C2], F32R, tag=f"wg{cc}")
                nc.sync.dma_start(t[:], wg_d[cc])
                wg.append(t)
            wo_f = constp.tile([C2, C], F32, tag="wo_f")
            nc.sync.dma_start(wo_f[:], wo_d[:])
            id_f = constp.tile([128, 128], F32, tag="id_f")
            nc.sync.dma_start(id_f[:], id_d[:])

            wo = constp.tile([C2, C], BF16, tag="wo")
            nc.vector.tensor_copy(wo[:], wo_f[:])
            id_b = constp.tile([128, 128], BF16, tag="id_b")
            nc.vector.tensor_copy(id_b[:], id_f[:])
            ones = constp.tile([128, 128], BF16, tag="ones")
            nc.gpsimd.memset(ones[:], 1.0)

            for b in range(BPC):
                # ---- load x ----
                xs = []
                for cc in range(2):
                    xt = xp.tile([128, N], F32R, tag="x")
                    nc.sync.dma_start(xt[:], x_d[b, cc * 128:(cc + 1) * 128, :])
                    xs.append(xt)

                # ---- projections ----
                th = projp.tile([C8, N], F32R, tag="theta")
                ph = projp.tile([C8, N], F32, tag="phi")
                g = projp.tile([C2, N], BF16, tag="g")
                for i in range(NNCH):
                    sl = slice(i * NCH, (i + 1) * NCH)
                    tps = pss.tile([128, NCH], F32, tag="small")
                    for cc in range(2):
                        nc.tensor.matmul(
                            tps[:C8], wth[cc][:], xs[cc][:, sl],
                            start=(cc == 0), stop=(cc == 1),
                        )
                    nc.scalar.copy(th[:, sl], tps[:C8])
                    pps = pss.tile([128, NCH], F32, tag="small")
                    for cc in range(2):
                        nc.tensor.matmul(
                            pps[:C8], wph[cc][:], xs[cc][:, sl],
                            start=(cc == 0), stop=(cc == 1),
                        )
                    nc.scalar.copy(ph[:, sl], pps[:C8])
                    gps = pss.tile([128, NCH], F32, tag="small")
                    for cc in range(2):
                        nc.tensor.matmul(
                            gps[:C2], wg[cc][:], xs[cc][:, sl],
                            start=(cc == 0), stop=(cc == 1),
                        )
                    nc.vector.tensor_copy(g[:, sl], gps[:C2])

                # ---- 2x2 maxpool: [*, 64, 64] -> [*, 32, 32] ----
                # w-pairs: view (h, w2, 2), innermost reduced by pool_max
                php_t = projp.tile([C8, N // 2], F32, tag="phi_t")
                php = projp.tile([C8, M], F32, tag="phi_p")
                v = ph[:].rearrange("p (h w2 two) -> p h w2 two", h=H, w2=W // 2, two=2)
                nc.vector.tensor_max(php_t[:], v[:, :, :, 0], v[:, :, :, 1])
                v2 = php_t[:].rearrange("p (h2 hb w2) -> p h2 w2 hb", h2=H // 2, hb=2, w2=W // 2)
                nc.vector.tensor_max(php[:], v2[:, :, :, 0], v2[:, :, :, 1])
                php_r = projp.tile([C8, M], F32R, tag="phi_pr")
                nc.scalar.copy(php_r[:], php[:])

                gp_t = projp.tile([C2, N // 2], BF16, tag="g_t")
                gp = projp.tile([C2, M], BF16, tag="g_p")
                v = g[:].rearrange("p (h w2 two) -> p h w2 two", h=H, w2=W // 2, two=2)
                nc.vector.tensor_max(gp_t[:], v[:, :, :, 0], v[:, :, :, 1])
                v2 = gp_t[:].rearrange("p (h2 hb w2) -> p h2 w2 hb", h2=H // 2, hb=2, w2=W // 2)
                nc.vector.tensor_max(gp[:], v2[:, :, :, 0], v2[:, :, :, 1])

                # ---- gT: transpose pooled g into 8 [128m, 128c] chunks ----
                gts = []
                for mc in range(MC):
                    tp = pss.tile([128, 128], BF16, tag="small")
                    nc.tensor.transpose(tp[:], gp[:, mc * 128:(mc + 1) * 128], id_b[:])
                    gt = gtp.tile([128, 128], BF16, tag=f"gt{mc}")
                    nc.vector.tensor_copy(gt[:], tp[:])
                    gts.append(gt)

                # ---- scoresT + exp: [m, n] layout ----
                ets = []
                for mc in range(MC):
                    et = expp.tile([128, N], BF16, tag=f"expT{mc}")
                    ets.append(et)
                    for qt in range(4):
                        sps = psb.tile([128, 1024], F32, tag="big")
                        for r in range(2):
                            nsl = slice(qt * 1024 + r * 512, qt * 1024 + (r + 1) * 512)
                            nc.tensor.matmul(
                                sps[:, r * 512:(r + 1) * 512],
                                php_r[:, mc * 128:(mc + 1) * 128],
                                th[:, nsl],
                                start=True, stop=True,
                            )
                        nc.scalar.activation(
                            et[:, qt * 1024:(qt + 1) * 1024], sps[:],
                            mybir.ActivationFunctionType.Exp,
                        )

                # ---- attention + denominator + normalize + output proj ----
                for i in range(NNCH):
                    nsl = slice(i * NCH, (i + 1) * NCH)
                    adps = psb.tile([128, 1024], F32, tag="big")
                    aps = adps[:, 0:NCH]
                    dps = adps[:, NCH:2 * NCH]
                    for mc in range(MC):
                        nc.tensor.matmul(
                            aps, gts[mc][:], ets[mc][:, nsl],
                            start=(mc == 0), stop=(mc == MC - 1),
                        )
                    for mc in range(MC):
                        nc.tensor.matmul(
                            dps, ones[:], ets[mc][:, nsl],
                            start=(mc == 0), stop=(mc == MC - 1),
                        )
                    rec = smallp.tile([128, NCH], F32, tag="rec")
                    nc.vector.reciprocal_approx_fast(rec[:], dps)
                    at = smallp.tile([128, NCH], BF16, tag="attn")
                    nc.vector.scalar_tensor_tensor(
                        at[:], aps, 1.0, rec[:],
                        mybir.AluOpType.bypass, mybir.AluOpType.mult,
                    )
                    for oc in range(2):
                        ops = pss.tile([128, NCH], F32, tag="small")
                        nc.tensor.matmul(
                            ops[:], wo[:, oc * 128:(oc + 1) * 128], at[:],
                            start=True, stop=True,
                        )
                        osb = outp.tile([128, NCH], F32, tag="osb")
                        nc.vector.scalar_tensor_tensor(
                            osb[:], ops[:], 1.0, xs[oc][:, nsl].bitcast(F32),
                            mybir.AluOpType.bypass, mybir.AluOpType.add,
                        )
                        nc.sync.dma_start(out_d[b, oc * 128:(oc + 1) * 128, nsl], osb[:])

    nc.compile()
    return nc


_NC_CACHE = None


def _get_nc():
    global _NC_CACHE
    if _NC_CACHE is None:
        _NC_CACHE = build_kernel()
    return _NC_CACHE


def prep_inputs(x, w_theta, w_phi, w_g, w_o, gamma):
    """Host-side prep: shard x over 8 cores; transpose/scale weights."""
    x = np.asarray(x, dtype=np.float32).reshape(B, C, N)
    w_theta = np.asarray(w_theta, dtype=np.float32)
    w_phi = np.asarray(w_phi, dtype=np.float32)
    w_g = np.asarray(w_g, dtype=np.float32)
    w_o = np.asarray(w_o, dtype=np.float32)
    gamma = np.float32(gamma)

    wth = np.ascontiguousarray(w_theta.T.reshape(2, 128, C8))
    wph = np.ascontiguousarray(w_phi.T.reshape(2, 128, C8))
    wg = np.ascontiguousarray(w_g.T.reshape(2, 128, C2))
    wo = np.ascontiguousarray((gamma * w_o).T)
    ident = np.eye(128, dtype=np.float32)

    in_maps = []
    for core in range(NCORES):
        shard = np.ascontiguousarray(x[core * BPC:(core + 1) * BPC])
        in_maps.append({
            "x": shard, "wth": wth, "wph": wph, "wg": wg, "wo": wo,
            "ident": ident,
        })
    return in_maps


def run(inputs, trace=False, **kw):
    nc = _get_nc()
    in_maps = prep_inputs(**inputs)
    res = run_bass_kernel_spmd(nc, in_maps, core_ids=list(range(NCORES)),
                               trace=trace, **kw)
    outs = [res.results[i]["out"] for i in range(NCORES)]
    full = np.concatenate(outs, axis=0).reshape(B, C, H, W).astype(np.float32)
    return full, res


def kernel(**inputs):
    full, _ = run(inputs, trace=False)
    return full


# revision 12
# speedup vs baseline: 1.0925x; 1.0925x over previous
"""Self-attention (SAGAN-style) Trainium2 kernel.

Reference computation (per batch sample):
    theta = w_theta @ x            # [32, 4096]
    phi   = pool2x2(w_phi @ x)     # [32, 1024]
    g     = pool2x2(w_g @ x)       # [128, 1024]
    beta  = softmax(theta.T @ phi, axis=-1)   # [4096, 1024]
    attn  = g @ beta.T             # [128, 4096]
    out   = gamma * (w_o @ attn) + x

Sharding: data-parallel over batch; B=16 over 8 cores -> 2 samples/core.

Kernel strategy (per core, per sample), all matmuls bf16 (fp32 PSUM accum):
  - x cast to bf16 once; one combined projection weight [256, 128] computes
    theta twice and phi twice (rows 0:32/32:64 theta, 64:96/96:128 phi) so the
    K=32 score matmuls can run 2-way row-tiled (tile_position (0,0)/(32,0)).
  - maxpool 2x2: the w-pair max is fused into PSUM evacuation (strided
    tensor_max straight out of PSUM), h-pair max is one strided tensor_max.
  - scoresT in [m, n] layout; exp on ScalarE straight out of PSUM ->
    bf16 SBUF (logits are O(+-40): exp without max-subtraction is safe).
  - attn[c, n] = sum_mc gT[mc].T @ expT[mc]; gT from PE transposes of pooled
    g. The softmax denominator rides the same rhs streams through an all-ones
    stationary operand, which also broadcasts the row-sum to all partitions.
  - normalize via reciprocal_approx_fast + scalar_tensor_tensor;
    o = (gamma*w_o).T @ attn; residual fused into PSUM evacuation.
"""

import numpy as np

import concourse.bacc as bacc
import concourse.mybir as mybir
from concourse import tile
from concourse.bass_utils import run_bass_kernel_spmd

F32 = mybir.dt.float32
BF16 = mybir.dt.bfloat16

B, C, H, W = 16, 256, 64, 64
N = H * W            # 4096
M = N // 4           # 1024
C8 = C // 8          # 32
C2 = C // 2          # 128
NCORES = 8
BPC = B // NCORES    # 2 samples per core
NCH = 512            # n-chunk width for matmul streaming
NNCH = N // NCH      # 8
MC = M // 128        # 8 m-chunks


def build_kernel():
    nc = bacc.Bacc("TRN2", target_bir_lowering=False, debug=False)

    x_d = nc.declare_dram_parameter("x", [BPC, C, N], F32, isOutput=False)
    # [cc][128 chans][th th ph ph] and [cc][128 chans][g]
    wq_d = nc.declare_dram_parameter("wq", [2, 128, 128], F32, isOutput=False)
    wg_d = nc.declare_dram_parameter("wg", [2, 128, C2], F32, isOutput=False)
    wo_d = nc.declare_dram_parameter("wo", [C2, C], F32, isOutput=False)  # (gamma*w_o).T
    id_d = nc.declare_dram_parameter("ident", [128, 128], F32, isOutput=False)
    out_d = nc.declare_dram_parameter("out", [BPC, C, N], F32, isOutput=True)

    with tile.TileContext(nc) as tc:
        with (
            tc.tile_pool(name="const", bufs=1) as constp,
            tc.tile_pool(name="xin", bufs=2) as xp,
            tc.tile_pool(name="xbf", bufs=2) as xbfp,
            tc.tile_pool(name="proj", bufs=1) as projp,
            tc.tile_pool(name="exp", bufs=1) as expp,
            tc.tile_pool(name="gt", bufs=1) as gtp,
            tc.tile_pool(name="small", bufs=3) as smallp,
            tc.tile_pool(name="outs", bufs=4) as outp,
            tc.tile_pool(name="ps_big", bufs=3, space="PSUM") as psb,
            tc.tile_pool(name="ps_small", bufs=2, space="PSUM") as pss,
            # psb: tag "big" [128,1024] x3 slots = 6 banks
            # pss: tag "small" [128,512] x2 slots = 2 banks
        ):
            # ---- constants / weights (loaded once, cast to bf16) ----
            wq, wg = [], []
            for cc in range(2):
                tf = constp.tile([128, 128], F32, tag=f"wqf{cc}")
                nc.sync.dma_start(tf[:], wq_d[cc])
                t = constp.tile([128, 128], BF16, tag=f"wq{cc}")
                nc.vector.tensor_copy(t[:], tf[:])
                wq.append(t)
                tf2 = constp.tile([128, C2], F32, tag=f"wgf{cc}")
                nc.sync.dma_start(tf2[:], wg_d[cc])
                t = constp.tile([128, C2], BF16, tag=f"wg{cc}")
                nc.vector.tensor_copy(t[:], tf2[:])
                wg.append(t)
            wo_f = constp.tile([C2, C], F32, tag="wo_f")
            nc.sync.dma_start(wo_f[:], wo_d[:])
            wo = constp.tile([C2, C], BF16, tag="wo")
            nc.vector.tensor_copy(wo[:], wo_f[:])
            id_f = constp.tile([128, 128], F32, tag="id_f")
            nc.sync.dma_start(id_f[:], id_d[:])
            id_b = constp.tile([128, 128], BF16, tag="id_b")
            nc.vector.tensor_copy(id_b[:], id_f[:])
            ones = constp.tile([128, 128], BF16, tag="ones")
            nc.gpsimd.memset(ones[:], 1.0)

            for b in range(BPC):
                # ---- load x, cast to bf16 ----
                xs, xbs = [], []
                for cc in range(2):
                    xt = xp.tile([128, N], F32, tag="x")
                    nc.sync.dma_start(xt[:], x_d[b, cc * 128:(cc + 1) * 128, :])
                    xs.append(xt)
                    xb = xbfp.tile([128, N], BF16, tag="xb")
                    nc.vector.tensor_copy(xb[:], xt[:])
                    xbs.append(xb)

                # ---- projections ----
                thph = projp.tile([128, N], BF16, tag="thph")  # 0:64 dup-theta, 64:128 dup-phi
                g_sb = projp.tile([C2, N], BF16, tag="g_sb")
                for i in range(NNCH):
                    sl = slice(i * NCH, (i + 1) * NCH)
                    ps1 = pss.tile([128, NCH], F32, tag="small")
                    for cc in range(2):
                        nc.tensor.matmul(ps1[:], wq[cc][:], xbs[cc][:, sl],
                                         start=(cc == 0), stop=(cc == 1))
                    nc.scalar.copy(thph[:, sl], ps1[:])
                    ps2 = pss.tile([128, NCH], F32, tag="small")
                    for cc in range(2):
                        nc.tensor.matmul(ps2[:], wg[cc][:], xbs[cc][:, sl],
                                         start=(cc == 0), stop=(cc == 1))
                    nc.vector.tensor_copy(g_sb[:, sl], ps2[:])
                th2 = thph[0:64]

                # ---- 2x2 maxpool (w-pairs then h-pairs, strided SBUF ops) ----
                ph2t = projp.tile([64, N // 2], BF16, tag="ph2t")
                pv = thph[:].rearrange("p (w2 two) -> p w2 two", two=2)
                nc.vector.tensor_max(ph2t[:], pv[64:128, :, 0], pv[64:128, :, 1])
                ph2 = projp.tile([64, M], BF16, tag="ph2")
                v2 = ph2t[:].rearrange("p (h2 hb w2) -> p h2 w2 hb", h2=H // 2, hb=2, w2=W // 2)
                nc.vector.tensor_max(ph2[:], v2[:, :, :, 0], v2[:, :, :, 1])
                g_t = projp.tile([C2, N // 2], BF16, tag="g_t")
                pv2 = g_sb[:].rearrange("p (w2 two) -> p w2 two", two=2)
                nc.vector.tensor_max(g_t[:], pv2[:, :, 0], pv2[:, :, 1])
                gp = projp.tile([C2, M], BF16, tag="g_p")
                v2 = g_t[:].rearrange("p (h2 hb w2) -> p h2 w2 hb", h2=H // 2, hb=2, w2=W // 2)
                nc.vector.tensor_max(gp[:], v2[:, :, :, 0], v2[:, :, :, 1])

                # ---- gT: transpose pooled g into 8 [128m, 128c] chunks ----
                gts = []
                for mc in range(MC):
                    tp = pss.tile([128, 128], BF16, tag="small")
                    nc.tensor.transpose(tp[:], gp[:, mc * 128:(mc + 1) * 128], id_b[:])
                    gt = gtp.tile([128, 128], BF16, tag=f"gt{mc}")
                    nc.vector.tensor_copy(gt[:], tp[:])
                    gts.append(gt)

                # ---- scoresT + exp, 2-way row-tiled over mc pairs ----
                ets = []
                for mc in range(MC):
                    et = expp.tile([128, N], BF16, tag=f"expT{mc}", name=f"expT{mc}_{b}")
                    ets.append(et)
                for qt in range(4):
                    qsl = slice(qt * 1024, (qt + 1) * 1024)
                    for r in range(4):
                        mc_a, mc_b = 2 * r, 2 * r + 1
                        spa = psb.tile([128, 1024], F32, tag="big")
                        spb = psb.tile([128, 1024], F32, tag="big")
                        for hf in range(2):
                            nsl = slice(qt * 1024 + hf * 512, qt * 1024 + (hf + 1) * 512)
                            osl = slice(hf * 512, (hf + 1) * 512)
                            nc.tensor.matmul(
                                spa[:, osl], ph2[0:32, mc_a * 128:(mc_a + 1) * 128],
                                th2[0:32, nsl], start=True, stop=True)
                            nc.tensor.matmul(
                                spb[:, osl], ph2[32:64, mc_b * 128:(mc_b + 1) * 128],
                                th2[32:64, nsl], start=True, stop=True)
                        nc.scalar.activation(ets[mc_a][:, qsl], spa[:],
                                             mybir.ActivationFunctionType.Exp)
                        nc.scalar.activation(ets[mc_b][:, qsl], spb[:],
                                             mybir.ActivationFunctionType.Exp)

                # ---- attention + denominator + normalize + output proj ----
                for i in range(NNCH):
                    nsl = slice(i * NCH, (i + 1) * NCH)
                    adps = psb.tile([128, 1024], F32, tag="big")
                    aps = adps[:, 0:NCH]
                    dps = adps[:, NCH:2 * NCH]
                    for mc in range(MC):
                        nc.tensor.matmul(aps, gts[mc][:], ets[mc][:, nsl],
                                         start=(mc == 0), stop=(mc == MC - 1))
                    for mc in range(MC):
                        nc.tensor.matmul(dps, ones[:], ets[mc][:, nsl],
                                         start=(mc == 0), stop=(mc == MC - 1))
                    rec = smallp.tile([128, NCH], F32, tag="rec")
                    nc.vector.reciprocal_approx_fast(rec[:], dps)
                    at = smallp.tile([128, NCH], BF16, tag="attn")
                    nc.vector.scalar_tensor_tensor(
                        at[:], aps, 1.0, rec[:],
                        mybir.AluOpType.bypass, mybir.AluOpType.mult)
                    for oc in range(2):
                        ops = pss.tile([128, NCH], F32, tag="small")
                        nc.tensor.matmul(ops[:], wo[:, oc * 128:(oc + 1) * 128], at[:],
                                         start=True, stop=True)
                        osb = outp.tile([128, NCH], F32, tag="osb")
                        nc.vector.scalar_tensor_tensor(
                            osb[:], ops[:], 1.0, xs[oc][:, nsl],
                            mybir.AluOpType.bypass, mybir.AluOpType.add)
                        nc.sync.dma_start(out_d[b, oc * 128:(oc + 1) * 128, nsl], osb[:])

    nc.compile()
    return nc


_NC_CACHE = None


def _get_nc():
    global _NC_CACHE
    if _NC_CACHE is None:
        _NC_CACHE = build_kernel()
    return _NC_CACHE


def prep_inputs(x, w_theta, w_phi, w_g, w_o, gamma):
    """Host-side prep: shard x over 8 cores; transpose/scale/pack weights."""
    x = np.asarray(x, dtype=np.float32).reshape(B, C, N)
    w_theta = np.asarray(w_theta, dtype=np.float32)
    w_phi = np.asarray(w_phi, dtype=np.float32)
    w_g = np.asarray(w_g, dtype=np.float32)
    w_o = np.asarray(w_o, dtype=np.float32)
    gamma = np.float32(gamma)

    # combined projection weight: [th th ph ph] along output dim
    wqT = np.concatenate([w_theta.T, w_theta.T, w_phi.T, w_phi.T], axis=1)  # [256,128]
    wq = np.ascontiguousarray(wqT.reshape(2, 128, 128))
    wgq = np.ascontiguousarray(w_g.T.reshape(2, 128, C2))
    wo = np.ascontiguousarray((gamma * w_o).T)
    ident = np.eye(128, dtype=np.float32)

    in_maps = []
    for core in range(NCORES):
        shard = np.ascontiguousarray(x[core * BPC:(core + 1) * BPC])
        in_maps.append({"x": shard, "wq": wq, "wg": wgq, "wo": wo, "ident": ident})
    return in_maps


def run(inputs, trace=False, **kw):
    nc = _get_nc()
    in_maps = prep_inputs(**inputs)
    res = run_bass_kernel_spmd(nc, in_maps, core_ids=list(range(NCORES)),
                               trace=trace, **kw)
    outs = [res.results[i]["out"] for i in range(NCORES)]
    full = np.concatenate(outs, axis=0).reshape(B, C, H, W).astype(np.float32)
    return full, res


def kernel(**inputs):
    full, _ = run(inputs, trace=False)
    return full


# revision 13
# speedup vs baseline: 1.1391x; 1.0426x over previous
"""Self-attention (SAGAN-style) Trainium2 kernel.

Reference computation (per batch sample):
    theta = w_theta @ x            # [32, 4096]
    phi   = pool2x2(w_phi @ x)     # [32, 1024]
    g     = pool2x2(w_g @ x)       # [128, 1024]
    beta  = softmax(theta.T @ phi, axis=-1)   # [4096, 1024]
    attn  = g @ beta.T             # [128, 4096]
    out   = gamma * (w_o @ attn) + x

Sharding: data-parallel over batch; B=16 over 8 cores -> 2 samples/core.

Kernel strategy (per core, per sample), all matmuls bf16 (fp32 PSUM accum):
  - x loaded via gpsimd casting DMA straight to bf16; the fp32 x needed for
    the residual is re-DMAed in [128,512] chunks at consume time.
  - one combined projection weight [256, 128] computes theta twice and phi
    twice (rows 0:32/32:64 theta, 64:96/96:128 phi) so the K=32 score matmuls
    can run 2-way row-tiled (tile_position (0,0)/(32,0)).
  - scoresT in [m, n] layout; exp on ScalarE straight out of PSUM -> bf16
    SBUF (logits are O(+-40): exp without max-subtraction is safe). The
    score/exp work for quarter qt is emitted together with the attention
    for quarter qt-1 so the PE has matmul work while ScalarE exps.
  - attn[c, n] = sum_mc gT[mc].T @ expT[mc]; gT from PE transposes of pooled
    g. The softmax denominator rides the same rhs streams through an all-ones
    stationary operand, which also broadcasts the row-sum to all partitions.
  - normalize via reciprocal_approx_fast + scalar_tensor_tensor;
    o = (gamma*w_o).T @ attn; residual fused into PSUM evacuation.
"""

import numpy as np

import concourse.bacc as bacc
import concourse.mybir as mybir
from concourse import tile
from concourse.bass_utils import run_bass_kernel_spmd

F32 = mybir.dt.float32
BF16 = mybir.dt.bfloat16

B, C, H, W = 16, 256, 64, 64
N = H * W            # 4096
M = N // 4           # 1024
C8 = C // 8          # 32
C2 = C // 2          # 128
NCORES = 8
BPC = B // NCORES    # 2 samples per core
NCH = 512            # n-chunk width for matmul streaming
NNCH = N // NCH      # 8
MC = M // 128        # 8 m-chunks


def build_kernel():
    nc = bacc.Bacc("TRN2", target_bir_lowering=False, debug=False)

    x_d = nc.declare_dram_parameter("x", [BPC, C, N], F32, isOutput=False)
    # [cc][128 chans][th th ph ph] and [cc][128 chans][g]
    wq_d = nc.declare_dram_parameter("wq", [2, 128, 128], F32, isOutput=False)
    wg_d = nc.declare_dram_parameter("wg", [2, 128, C2], F32, isOutput=False)
    wo_d = nc.declare_dram_parameter("wo", [C2, C], F32, isOutput=False)  # (gamma*w_o).T
    id_d = nc.declare_dram_parameter("ident", [128, 128], F32, isOutput=False)
    out_d = nc.declare_dram_parameter("out", [BPC, C, N], F32, isOutput=True)

    with tile.TileContext(nc) as tc:
        with (
            tc.tile_pool(name="const", bufs=1) as constp,
            tc.tile_pool(name="xbf", bufs=4) as xbfp,
            tc.tile_pool(name="xres", bufs=6) as xrp,
            tc.tile_pool(name="proj", bufs=2) as projp,
            tc.tile_pool(name="exp", bufs=1) as expp,
            tc.tile_pool(name="gt", bufs=1) as gtp,
            tc.tile_pool(name="small", bufs=3) as smallp,
            tc.tile_pool(name="outs", bufs=4) as outp,
            tc.tile_pool(name="ps_big", bufs=3, space="PSUM") as psb,
            tc.tile_pool(name="ps_small", bufs=2, space="PSUM") as pss,
        ):
            # ---- constants / weights (loaded once, cast by DMA) ----
            wq, wg = [], []
            for cc in range(2):
                t = constp.tile([128, 128], BF16, tag=f"wq{cc}")
                nc.gpsimd.dma_start(t[:], wq_d[cc])
                wq.append(t)
                t = constp.tile([128, C2], BF16, tag=f"wg{cc}")
                nc.gpsimd.dma_start(t[:], wg_d[cc])
                wg.append(t)
            wo = constp.tile([C2, C], BF16, tag="wo")
            nc.gpsimd.dma_start(wo[:], wo_d[:])
            id_b = constp.tile([128, 128], BF16, tag="id_b")
            nc.gpsimd.dma_start(id_b[:], id_d[:])
            ones = constp.tile([128, 128], BF16, tag="ones")
            nc.gpsimd.memset(ones[:], 1.0)

            for b in range(BPC):
                # ---- load x as bf16 (casting DMA on gpsimd SWDGE) ----
                xbs = []
                for cc in range(2):
                    xb = xbfp.tile([128, N], BF16, tag="xb", name=f"xb{b}_{cc}")
                    nc.gpsimd.dma_start(xb[:], x_d[b, cc * 128:(cc + 1) * 128, :])
                    xbs.append(xb)

                # ---- projections ----
                thph = projp.tile([128, N], BF16, tag="thph")  # 0:64 dup-theta, 64:128 dup-phi
                g_sb = projp.tile([C2, N], BF16, tag="g_sb")
                for i in range(NNCH):
                    sl = slice(i * NCH, (i + 1) * NCH)
                    ps1 = pss.tile([128, NCH], F32, tag="small", name=f"ps1_{b}_{i}")
                    for cc in range(2):
                        nc.tensor.matmul(ps1[:], wq[cc][:], xbs[cc][:, sl],
                                         start=(cc == 0), stop=(cc == 1))
                    nc.vector.tensor_copy(thph[:, sl], ps1[:])
                    ps2 = pss.tile([128, NCH], F32, tag="small", name=f"ps2_{b}_{i}")
                    for cc in range(2):
                        nc.tensor.matmul(ps2[:], wg[cc][:], xbs[cc][:, sl],
                                         start=(cc == 0), stop=(cc == 1))
                    nc.vector.tensor_copy(g_sb[:, sl], ps2[:])
                th2 = thph[0:64]

                # ---- 2x2 maxpool (w-pairs then h-pairs, strided SBUF ops) ----
                ph2t = projp.tile([64, N // 2], BF16, tag="ph2t")
                pv = thph[:].rearrange("p (w2 two) -> p w2 two", two=2)
                nc.vector.tensor_max(ph2t[:], pv[64:128, :, 0], pv[64:128, :, 1])
                ph2 = projp.tile([64, M], BF16, tag="ph2")
                v2 = ph2t[:].rearrange("p (h2 hb w2) -> p h2 w2 hb", h2=H // 2, hb=2, w2=W // 2)
                nc.vector.tensor_max(ph2[:], v2[:, :, :, 0], v2[:, :, :, 1])
                g_t = projp.tile([C2, N // 2], BF16, tag="g_t")
                pv2 = g_sb[:].rearrange("p (w2 two) -> p w2 two", two=2)
                nc.vector.tensor_max(g_t[:], pv2[:, :, 0], pv2[:, :, 1])
                gp = projp.tile([C2, M], BF16, tag="g_p")
                v2 = g_t[:].rearrange("p (h2 hb w2) -> p h2 w2 hb", h2=H // 2, hb=2, w2=W // 2)
                nc.vector.tensor_max(gp[:], v2[:, :, :, 0], v2[:, :, :, 1])

                # ---- gT: transpose pooled g into 8 [128m, 128c] chunks ----
                gts = []
                for mc in range(MC):
                    tp = pss.tile([128, 128], BF16, tag="small", name=f"tp{b}_{mc}")
                    nc.tensor.transpose(tp[:], gp[:, mc * 128:(mc + 1) * 128], id_b[:])
                    gt = gtp.tile([128, 128], BF16, tag=f"gt{mc}", name=f"gt{mc}_{b}")
                    nc.vector.tensor_copy(gt[:], tp[:])
                    gts.append(gt)

                # ---- scores/exp (quarter qt) interleaved with attention (qt-1) ----
                ets = []
                for mc in range(MC):
                    et = expp.tile([128, N], BF16, tag=f"expT{mc}", name=f"expT{mc}_{b}")
                    ets.append(et)

                def attn_chunk(i, b=b, ets=ets, gts=gts):
                    nsl = slice(i * NCH, (i + 1) * NCH)
                    xr = []
                    for oc in range(2):
                        xt = xrp.tile([128, NCH], F32, tag="xr",
                                      name=f"xr{b}_{i}_{oc}")
                        nc.sync.dma_start(xt[:], x_d[b, oc * 128:(oc + 1) * 128, nsl])
                        xr.append(xt)
                    adps = psb.tile([128, 1024], F32, tag="big", name=f"adps{b}_{i}")
                    aps = adps[:, 0:NCH]
                    dps = adps[:, NCH:2 * NCH]
                    for mc in range(MC):
                        nc.tensor.matmul(aps, gts[mc][:], ets[mc][:, nsl],
                                         start=(mc == 0), stop=(mc == MC - 1))
                    for mc in range(MC):
                        nc.tensor.matmul(dps, ones[:], ets[mc][:, nsl],
                                         start=(mc == 0), stop=(mc == MC - 1))
                    rec = smallp.tile([128, NCH], F32, tag="rec", name=f"rec{b}_{i}")
                    nc.vector.reciprocal_approx_fast(rec[:], dps)
                    at = smallp.tile([128, NCH], BF16, tag="attn", name=f"at{b}_{i}")
                    nc.vector.scalar_tensor_tensor(
                        at[:], aps, 1.0, rec[:],
                        mybir.AluOpType.bypass, mybir.AluOpType.mult)
                    for oc in range(2):
                        ops = pss.tile([128, NCH], F32, tag="small",
                                       name=f"ops{b}_{i}_{oc}")
                        nc.tensor.matmul(ops[:], wo[:, oc * 128:(oc + 1) * 128], at[:],
                                         start=True, stop=True)
                        osb = outp.tile([128, NCH], F32, tag="osb",
                                        name=f"osb{b}_{i}_{oc}")
                        nc.vector.scalar_tensor_tensor(
                            osb[:], ops[:], 1.0, xr[oc][:],
                            mybir.AluOpType.bypass, mybir.AluOpType.add)
                        nc.sync.dma_start(out_d[b, oc * 128:(oc + 1) * 128, nsl], osb[:])

                for qt in range(5):
                    if qt < 4:
                        qsl = slice(qt * 1024, (qt + 1) * 1024)
                        for r in range(4):
                            mc_a, mc_b = 2 * r, 2 * r + 1
                            spa = psb.tile([128, 1024], F32, tag="big",
                                           name=f"spa{b}_{qt}_{r}")
                            spb = psb.tile([128, 1024], F32, tag="big",
                                           name=f"spb{b}_{qt}_{r}")
                            for hf in range(2):
                                nsl = slice(qt * 1024 + hf * 512, qt * 1024 + (hf + 1) * 512)
                                osl = slice(hf * 512, (hf + 1) * 512)
                                nc.tensor.matmul(
                                    spa[:, osl], ph2[0:32, mc_a * 128:(mc_a + 1) * 128],
                                    th2[0:32, nsl], start=True, stop=True)
                                nc.tensor.matmul(
                                    spb[:, osl], ph2[32:64, mc_b * 128:(mc_b + 1) * 128],
                                    th2[32:64, nsl], start=True, stop=True)
                            nc.scalar.activation(ets[mc_a][:, qsl], spa[:],
                                                 mybir.ActivationFunctionType.Exp)
                            nc.scalar.activation(ets[mc_b][:, qsl], spb[:],
                                                 mybir.ActivationFunctionType.Exp)
                    if qt >= 1:
                        attn_chunk(2 * (qt - 1))
                        attn_chunk(2 * (qt - 1) + 1)

    nc.compile()
    return nc


_NC_CACHE = None


def _get_nc():
    global _NC_CACHE
    if _NC_CACHE is None:
        _NC_CACHE = build_kernel()
    return _NC_CACHE


def prep_inputs(x, w_theta, w_phi, w_g, w_o, gamma):
    """Host-side prep: shard x over 8 cores; transpose/scale/pack weights."""
    x = np.asarray(x, dtype=np.float32).reshape(B, C, N)
    w_theta = np.asarray(w_theta, dtype=np.float32)
    w_phi = np.asarray(w_phi, dtype=np.float32)
    w_g = np.asarray(w_g, dtype=np.float32)
    w_o = np.asarray(w_o, dtype=np.float32)
    gamma = np.float32(gamma)

    # combined projection weight: [th th ph ph] along output dim
    wqT = np.concatenate([w_theta.T, w_theta.T, w_phi.T, w_phi.T], axis=1)  # [256,128]
    wq = np.ascontiguousarray(wqT.reshape(2, 128, 128))
    wgq = np.ascontiguousarray(w_g.T.reshape(2, 128, C2))
    wo = np.ascontiguousarray((gamma * w_o).T)
    ident = np.eye(128, dtype=np.float32)

    in_maps = []
    for core in range(NCORES):
        shard = np.ascontiguousarray(x[core * BPC:(core + 1) * BPC])
        in_maps.append({"x": shard, "wq": wq, "wg": wgq, "wo": wo, "ident": ident})
    return in_maps


def run(inputs, trace=False, **kw):
    nc = _get_nc()
    in_maps = prep_inputs(**inputs)
    res = run_bass_kernel_spmd(nc, in_maps, core_ids=list(range(NCORES)),
                               trace=trace, **kw)
    outs = [res.results[i]["out"] for i in range(NCORES)]
    full = np.concatenate(outs, axis=0).reshape(B, C, H, W).astype(np.float32)
    return full, res


def kernel(**inputs):
    full, _ = run(inputs, trace=False)
    return full


# revision 15
# speedup vs baseline: 1.1440x; 1.0043x over previous
"""Self-attention (SAGAN-style) Trainium2 kernel.

Reference computation (per batch sample):
    theta = w_theta @ x            # [32, 4096]
    phi   = pool2x2(w_phi @ x)     # [32, 1024]
    g     = pool2x2(w_g @ x)       # [128, 1024]
    beta  = softmax(theta.T @ phi, axis=-1)   # [4096, 1024]
    attn  = g @ beta.T             # [128, 4096]
    out   = gamma * (w_o @ attn) + x

Sharding: data-parallel over batch; B=16 over 8 cores -> 2 samples/core.

Kernel strategy (per core, per sample), all matmuls bf16 (fp32 PSUM accum):
  - x loaded via gpsimd casting DMA straight to bf16; the fp32 x needed for
    the residual is re-DMAed in [128,512] chunks at consume time.
  - one combined projection weight [256, 128] computes theta twice and phi
    twice (rows 0:32/32:64 theta, 64:96/96:128 phi) so the K=32 score matmuls
    can run 2-way row-tiled (tile_position (0,0)/(32,0)).
  - scoresT in [m, n] layout; exp on ScalarE straight out of PSUM -> bf16
    SBUF (logits are O(+-40): exp without max-subtraction is safe). The
    score/exp work for quarter qt is emitted together with the attention
    for quarter qt-1 so the PE has matmul work while ScalarE exps.
  - attn[c, n] = sum_mc gT[mc].T @ expT[mc]; gT from PE transposes of pooled
    g. The softmax denominator rides the same rhs streams through an all-ones
    stationary operand, which also broadcasts the row-sum to all partitions.
  - normalize via reciprocal_approx_fast + scalar_tensor_tensor;
    o = (gamma*w_o).T @ attn; residual fused into PSUM evacuation.
"""

import numpy as np

import concourse.bacc as bacc
import concourse.mybir as mybir
from concourse import tile
from concourse.bass_utils import run_bass_kernel_spmd

F32 = mybir.dt.float32
BF16 = mybir.dt.bfloat16

B, C, H, W = 16, 256, 64, 64
N = H * W            # 4096
M = N // 4           # 1024
C8 = C // 8          # 32
C2 = C // 2          # 128
NCORES = 8
BPC = B // NCORES    # 2 samples per core
NCH = 512            # n-chunk width for matmul streaming
NNCH = N // NCH      # 8
MC = M // 128        # 8 m-chunks


def build_kernel():
    nc = bacc.Bacc("TRN2", target_bir_lowering=False, debug=False)

    x_d = nc.declare_dram_parameter("x", [BPC, C, N], F32, isOutput=False)
    # [cc][128 chans][th th ph ph] and [cc][128 chans][g]
    wq_d = nc.declare_dram_parameter("wq", [2, 128, 128], F32, isOutput=False)
    wg_d = nc.declare_dram_parameter("wg", [2, 128, C2], F32, isOutput=False)
    wo_d = nc.declare_dram_parameter("wo", [C2, C], F32, isOutput=False)  # (gamma*w_o).T
    id_d = nc.declare_dram_parameter("ident", [128, 128], F32, isOutput=False)
    out_d = nc.declare_dram_parameter("out", [BPC, C, N], F32, isOutput=True)

    with tile.TileContext(nc) as tc:
        with (
            tc.tile_pool(name="const", bufs=1) as constp,
            tc.tile_pool(name="xbf", bufs=4) as xbfp,
            tc.tile_pool(name="xres", bufs=6) as xrp,
            tc.tile_pool(name="proj", bufs=2) as projp,
            tc.tile_pool(name="exp", bufs=1) as expp,
            tc.tile_pool(name="gt", bufs=1) as gtp,
            tc.tile_pool(name="small", bufs=3) as smallp,
            tc.tile_pool(name="outs", bufs=4) as outp,
            tc.tile_pool(name="ps_big", bufs=2, space="PSUM") as psb,
            tc.tile_pool(name="ps_attn", bufs=2, space="PSUM") as psa,
        ):
            # ---- constants / weights (loaded once, cast by DMA) ----
            wq, wg = [], []
            for cc in range(2):
                t = constp.tile([128, 128], BF16, tag=f"wq{cc}")
                nc.gpsimd.dma_start(t[:], wq_d[cc])
                wq.append(t)
                t = constp.tile([128, C2], BF16, tag=f"wg{cc}")
                nc.gpsimd.dma_start(t[:], wg_d[cc])
                wg.append(t)
            wo = constp.tile([C2, C], BF16, tag="wo")
            nc.gpsimd.dma_start(wo[:], wo_d[:])
            id_b = constp.tile([128, 128], BF16, tag="id_b")
            nc.gpsimd.dma_start(id_b[:], id_d[:])
            ones = constp.tile([128, 128], BF16, tag="ones")
            nc.gpsimd.memset(ones[:], 1.0)

            for b in range(BPC):
                # ---- load x as bf16 (casting DMA on gpsimd SWDGE) ----
                xbs = []
                for cc in range(2):
                    xb = xbfp.tile([128, N], BF16, tag="xb", name=f"xb{b}_{cc}")
                    nc.gpsimd.dma_start(xb[:], x_d[b, cc * 128:(cc + 1) * 128, :])
                    xbs.append(xb)

                # ---- projections ----
                thph = projp.tile([128, N], BF16, tag="thph")  # 0:64 dup-theta, 64:128 dup-phi
                g_sb = projp.tile([C2, N], BF16, tag="g_sb")
                for i in range(NNCH):
                    sl = slice(i * NCH, (i + 1) * NCH)
                    ps1 = psa.tile([128, NCH], F32, tag="adps", name=f"ps1_{b}_{i}")
                    for cc in range(2):
                        nc.tensor.matmul(ps1[:], wq[cc][:], xbs[cc][:, sl],
                                         start=(cc == 0), stop=(cc == 1))
                    nc.vector.tensor_copy(thph[:, sl], ps1[:])
                    ps2 = psa.tile([128, NCH], F32, tag="adps", name=f"ps2_{b}_{i}")
                    for cc in range(2):
                        nc.tensor.matmul(ps2[:], wg[cc][:], xbs[cc][:, sl],
                                         start=(cc == 0), stop=(cc == 1))
                    nc.vector.tensor_copy(g_sb[:, sl], ps2[:])
                th2 = thph[0:64]

                # ---- 2x2 maxpool (w-pairs then h-pairs, strided SBUF ops) ----
                ph2t = projp.tile([64, N // 2], BF16, tag="ph2t")
                pv = thph[:].rearrange("p (w2 two) -> p w2 two", two=2)
                nc.vector.tensor_max(ph2t[:], pv[64:128, :, 0], pv[64:128, :, 1])
                ph2 = projp.tile([64, M], BF16, tag="ph2")
                v2 = ph2t[:].rearrange("p (h2 hb w2) -> p h2 w2 hb", h2=H // 2, hb=2, w2=W // 2)
                nc.vector.tensor_max(ph2[:], v2[:, :, :, 0], v2[:, :, :, 1])
                g_t = projp.tile([C2, N // 2], BF16, tag="g_t")
                pv2 = g_sb[:].rearrange("p (w2 two) -> p w2 two", two=2)
                nc.vector.tensor_max(g_t[:], pv2[:, :, 0], pv2[:, :, 1])
                gp = projp.tile([C2, M], BF16, tag="g_p")
                v2 = g_t[:].rearrange("p (h2 hb w2) -> p h2 w2 hb", h2=H // 2, hb=2, w2=W // 2)
                nc.vector.tensor_max(gp[:], v2[:, :, :, 0], v2[:, :, :, 1])

                # ---- gT: transpose pooled g into 8 [128m, 128c] chunks ----
                gts = []
                for mc in range(MC):
                    tp = psa.tile([128, 128], BF16, tag="adps", name=f"tp{b}_{mc}")
                    nc.tensor.transpose(tp[:], gp[:, mc * 128:(mc + 1) * 128], id_b[:])
                    gt = gtp.tile([128, 128], BF16, tag=f"gt{mc}", name=f"gt{mc}_{b}")
                    nc.vector.tensor_copy(gt[:], tp[:])
                    gts.append(gt)

                # ---- scores/exp interleaved with attention at round granularity ----
                # PE executes its stream in order, so alternate 4 score MMs
                # (one round) with 8 attention-accumulation MMs; ScalarE exps
                # overlap the attention matmuls.
                ets = []
                for mc in range(MC):
                    et = expp.tile([128, N], BF16, tag=f"expT{mc}", name=f"expT{mc}_{b}")
                    ets.append(et)

                # deferred attention work-units, 8 accumulation MMs each:
                # (chunk, 'attn') and (chunk, 'den') + epilogue after 'den'
                adps_map = {}

                def unit_attn(i):
                    nsl = slice(i * NCH, (i + 1) * NCH)
                    adps = psa.tile([128, 1024], F32, tag="adps", name=f"adps{b}_{i}")
                    adps_map[i] = adps
                    aps = adps[:, 0:NCH]
                    for mc in range(MC):
                        nc.tensor.matmul(aps, gts[mc][:], ets[mc][:, nsl],
                                         start=(mc == 0), stop=(mc == MC - 1),
                                         skip_group_check=True)

                def unit_den_epi(i):
                    nsl = slice(i * NCH, (i + 1) * NCH)
                    adps = adps_map.pop(i)
                    aps = adps[:, 0:NCH]
                    dps = adps[:, NCH:2 * NCH]
                    xr = xrp.tile([128, 1024], F32, tag="xr", name=f"xr{b}_{i}")
                    for oc in range(2):
                        nc.sync.dma_start(xr[:, oc * NCH:(oc + 1) * NCH],
                                          x_d[b, oc * 128:(oc + 1) * 128, nsl])
                    for mc in range(MC):
                        nc.tensor.matmul(dps, ones[:], ets[mc][:, nsl],
                                         start=(mc == 0), stop=(mc == MC - 1),
                                         skip_group_check=True)
                    rec = smallp.tile([128, NCH], F32, tag="rec", name=f"rec{b}_{i}")
                    nc.vector.reciprocal_approx_fast(rec[:], dps)
                    at = smallp.tile([128, NCH], BF16, tag="attn", name=f"at{b}_{i}")
                    nc.vector.scalar_tensor_tensor(
                        at[:], aps, 1.0, rec[:],
                        mybir.AluOpType.bypass, mybir.AluOpType.mult)
                    ops = psa.tile([128, 1024], F32, tag="adps", name=f"ops{b}_{i}")
                    for oc in range(2):
                        nc.tensor.matmul(ops[:, oc * NCH:(oc + 1) * NCH],
                                         wo[:, oc * 128:(oc + 1) * 128], at[:],
                                         start=True, stop=True)
                    osb = outp.tile([128, 1024], F32, tag="osb", name=f"osb{b}_{i}")
                    nc.vector.scalar_tensor_tensor(
                        osb[:], ops[:], 1.0, xr[:],
                        mybir.AluOpType.bypass, mybir.AluOpType.add)
                    for oc in range(2):
                        nc.sync.dma_start(out_d[b, oc * 128:(oc + 1) * 128, nsl],
                                          osb[:, oc * NCH:(oc + 1) * NCH])

                units = []
                for i in range(NNCH):
                    units.append(lambda i=i: unit_attn(i))
                    units.append(lambda i=i: unit_den_epi(i))
                uidx = 0

                for qt in range(5):
                    if qt < 4:
                        qsl = slice(qt * 1024, (qt + 1) * 1024)
                        for r in range(4):
                            mc_a, mc_b = 2 * r, 2 * r + 1
                            spa = psb.tile([128, 1024], F32, tag="big",
                                           name=f"spa{b}_{qt}_{r}")
                            spb = psb.tile([128, 1024], F32, tag="big",
                                           name=f"spb{b}_{qt}_{r}")
                            for hf in range(2):
                                nsl = slice(qt * 1024 + hf * 512, qt * 1024 + (hf + 1) * 512)
                                osl = slice(hf * 512, (hf + 1) * 512)
                                nc.tensor.matmul(
                                    spa[:, osl], ph2[0:32, mc_a * 128:(mc_a + 1) * 128],
                                    th2[0:32, nsl], start=True, stop=True)
                                nc.tensor.matmul(
                                    spb[:, osl], ph2[32:64, mc_b * 128:(mc_b + 1) * 128],
                                    th2[32:64, nsl], start=True, stop=True)
                            nc.scalar.activation(ets[mc_a][:, qsl], spa[:],
                                                 mybir.ActivationFunctionType.Exp)
                            nc.scalar.activation(ets[mc_b][:, qsl], spb[:],
                                                 mybir.ActivationFunctionType.Exp)
                            if qt >= 1 and uidx < len(units):
                                units[uidx](); uidx += 1
                    else:
                        while uidx < len(units):
                            units[uidx](); uidx += 1

    nc.compile()
    return nc


_NC_CACHE = None


def _get_nc():
    global _NC_CACHE
    if _NC_CACHE is None:
        _NC_CACHE = build_kernel()
    return _NC_CACHE


def prep_inputs(x, w_theta, w_phi, w_g, w_o, gamma):
    """Host-side prep: shard x over 8 cores; transpose/scale/pack weights."""
    x = np.asarray(x, dtype=np.float32).reshape(B, C, N)
    w_theta = np.asarray(w_theta, dtype=np.float32)
    w_phi = np.asarray(w_phi, dtype=np.float32)
    w_g = np.asarray(w_g, dtype=np.float32)
    w_o = np.asarray(w_o, dtype=np.float32)
    gamma = np.float32(gamma)

    # combined projection weight: [th th ph ph] along output dim
    wqT = np.concatenate([w_theta.T, w_theta.T, w_phi.T, w_phi.T], axis=1)  # [256,128]
    wq = np.ascontiguousarray(wqT.reshape(2, 128, 128))
    wgq = np.ascontiguousarray(w_g.T.reshape(2, 128, C2))
    wo = np.ascontiguousarray((gamma * w_o).T)
    ident = np.eye(128, dtype=np.float32)

    in_maps = []
    for core in range(NCORES):
        shard = np.ascontiguousarray(x[core * BPC:(core + 1) * BPC])
        in_maps.append({"x": shard, "wq": wq, "wg": wgq, "wo": wo, "ident": ident})
    return in_maps


def run(inputs, trace=False, **kw):
    nc = _get_nc()
    in_maps = prep_inputs(**inputs)
    res = run_bass_kernel_spmd(nc, in_maps, core_ids=list(range(NCORES)),
                               trace=trace, **kw)
    outs = [res.results[i]["out"] for i in range(NCORES)]
    full = np.concatenate(outs, axis=0).reshape(B, C, H, W).astype(np.float32)
    return full, res


def kernel(**inputs):
    full, _ = run(inputs, trace=False)
    return full


# revision 16
# speedup vs baseline: 1.2323x; 1.0772x over previous
"""Self-attention (SAGAN-style) Trainium2 kernel.

Reference computation (per batch sample):
    theta = w_theta @ x            # [32, 4096]
    phi   = pool2x2(w_phi @ x)     # [32, 1024]
    g     = pool2x2(w_g @ x)       # [128, 1024]
    beta  = softmax(theta.T @ phi, axis=-1)   # [4096, 1024]
    attn  = g @ beta.T             # [128, 4096]
    out   = gamma * (w_o @ attn) + x

Sharding: data-parallel over batch; B=16 over 8 cores -> 2 samples/core.

Kernel strategy (per core, per sample), all matmuls bf16 (fp32 PSUM accum):
  - x loaded via gpsimd casting DMA straight to bf16; the fp32 x needed for
    the residual is re-DMAed in [128,512] chunks at consume time.
  - one combined projection weight [256, 128] computes theta twice and phi
    twice (rows 0:32/32:64 theta, 64:96/96:128 phi) so the K=32 score matmuls
    can run 2-way row-tiled (tile_position (0,0)/(32,0)).
  - scoresT in [m, n] layout; exp on ScalarE straight out of PSUM -> bf16
    SBUF (logits are O(+-40): exp without max-subtraction is safe). The
    score/exp work for quarter qt is emitted together with the attention
    for quarter qt-1 so the PE has matmul work while ScalarE exps.
  - attn[c, n] = sum_mc gT[mc].T @ expT[mc]; gT from PE transposes of pooled
    g. The softmax denominator rides the same rhs streams through an all-ones
    stationary operand, which also broadcasts the row-sum to all partitions.
  - normalize via reciprocal_approx_fast + scalar_tensor_tensor;
    o = (gamma*w_o).T @ attn; residual fused into PSUM evacuation.
"""

import numpy as np

import concourse.bacc as bacc
import concourse.mybir as mybir
from concourse import tile
from concourse.bass_utils import run_bass_kernel_spmd

F32 = mybir.dt.float32
BF16 = mybir.dt.bfloat16

B, C, H, W = 16, 256, 64, 64
N = H * W            # 4096
M = N // 4           # 1024
C8 = C // 8          # 32
C2 = C // 2          # 128
NCORES = 8
BPC = B // NCORES    # 2 samples per core
NCH = 512            # n-chunk width for matmul streaming
NNCH = N // NCH      # 8
MC = M // 128        # 8 m-chunks


def build_kernel():
    nc = bacc.Bacc("TRN2", target_bir_lowering=False, debug=False)

    x_d = nc.declare_dram_parameter("x", [BPC, C, N], F32, isOutput=False)
    # [cc][128 chans][th th ph ph] and [cc][128 chans][g]
    wq_d = nc.declare_dram_parameter("wq", [2, 128, 128], F32, isOutput=False)
    wg_d = nc.declare_dram_parameter("wg", [2, 128, C2], F32, isOutput=False)
    wo_d = nc.declare_dram_parameter("wo", [C2, C], F32, isOutput=False)  # (gamma*w_o).T
    id_d = nc.declare_dram_parameter("ident", [128, 128], F32, isOutput=False)
    out_d = nc.declare_dram_parameter("out", [BPC, C, N], F32, isOutput=True)

    with tile.TileContext(nc) as tc:
        with (
            tc.tile_pool(name="const", bufs=1) as constp,
            tc.tile_pool(name="xbf", bufs=4) as xbfp,
            tc.tile_pool(name="xres", bufs=6) as xrp,
            tc.tile_pool(name="proj", bufs=2) as projp,
            tc.tile_pool(name="exp", bufs=1) as expp,
            tc.tile_pool(name="gt", bufs=1) as gtp,
            tc.tile_pool(name="small", bufs=3) as smallp,
            tc.tile_pool(name="outs", bufs=4) as outp,
            tc.tile_pool(name="ps_big", bufs=3, space="PSUM") as psb,
            tc.tile_pool(name="ps_a", bufs=1, space="PSUM") as psa,
            tc.tile_pool(name="ps_d", bufs=1, space="PSUM") as psd,
        ):
            # ---- constants / weights (loaded once, cast by DMA) ----
            wq, wg = [], []
            for cc in range(2):
                t = constp.tile([128, 128], BF16, tag=f"wq{cc}")
                nc.gpsimd.dma_start(t[:], wq_d[cc])
                wq.append(t)
                t = constp.tile([128, C2], BF16, tag=f"wg{cc}")
                nc.gpsimd.dma_start(t[:], wg_d[cc])
                wg.append(t)
            wo = constp.tile([C2, C], BF16, tag="wo")
            nc.gpsimd.dma_start(wo[:], wo_d[:])
            id_b = constp.tile([128, 128], BF16, tag="id_b")
            nc.gpsimd.dma_start(id_b[:], id_d[:])
            ones = constp.tile([128, 128], BF16, tag="ones")
            nc.gpsimd.memset(ones[:], 1.0)

            for b in range(BPC):
                # ---- load x as bf16 (casting DMA on gpsimd SWDGE) ----
                xbs = []
                for cc in range(2):
                    xb = xbfp.tile([128, N], BF16, tag="xb", name=f"xb{b}_{cc}")
                    nc.gpsimd.dma_start(xb[:], x_d[b, cc * 128:(cc + 1) * 128, :])
                    xbs.append(xb)

                # ---- projections ----
                thph = projp.tile([128, N], BF16, tag="thph")  # 0:64 dup-theta, 64:128 dup-phi
                g_sb = projp.tile([C2, N], BF16, tag="g_sb")
                for i in range(NNCH):
                    sl = slice(i * NCH, (i + 1) * NCH)
                    ps1 = psb.tile([128, NCH], F32, tag="big", name=f"ps1_{b}_{i}")
                    for cc in range(2):
                        nc.tensor.matmul(ps1[:], wq[cc][:], xbs[cc][:, sl],
                                         start=(cc == 0), stop=(cc == 1))
                    nc.scalar.copy(thph[:, sl], ps1[:])
                    ps2 = psb.tile([128, NCH], F32, tag="big", name=f"ps2_{b}_{i}")
                    for cc in range(2):
                        nc.tensor.matmul(ps2[:], wg[cc][:], xbs[cc][:, sl],
                                         start=(cc == 0), stop=(cc == 1))
                    nc.scalar.copy(g_sb[:, sl], ps2[:])
                th2 = thph[0:64]

                # ---- 2x2 maxpool (w-pairs then h-pairs, strided SBUF ops) ----
                ph2t = projp.tile([64, N // 2], BF16, tag="ph2t")
                pv = thph[:].rearrange("p (w2 two) -> p w2 two", two=2)
                nc.vector.tensor_max(ph2t[:], pv[64:128, :, 0], pv[64:128, :, 1])
                ph2 = projp.tile([64, M], BF16, tag="ph2")
                v2 = ph2t[:].rearrange("p (h2 hb w2) -> p h2 w2 hb", h2=H // 2, hb=2, w2=W // 2)
                nc.vector.tensor_max(ph2[:], v2[:, :, :, 0], v2[:, :, :, 1])
                g_t = projp.tile([C2, N // 2], BF16, tag="g_t")
                pv2 = g_sb[:].rearrange("p (w2 two) -> p w2 two", two=2)
                nc.vector.tensor_max(g_t[:], pv2[:, :, 0], pv2[:, :, 1])
                gp = projp.tile([C2, M], BF16, tag="g_p")
                v2 = g_t[:].rearrange("p (h2 hb w2) -> p h2 w2 hb", h2=H // 2, hb=2, w2=W // 2)
                nc.vector.tensor_max(gp[:], v2[:, :, :, 0], v2[:, :, :, 1])

                # ---- gT: transpose pooled g into 8 [128m, 128c] chunks ----
                gts = []
                for mc in range(MC):
                    tp = psa.tile([128, 128], BF16, tag="a", name=f"tp{b}_{mc}")
                    nc.tensor.transpose(tp[:], gp[:, mc * 128:(mc + 1) * 128], id_b[:])
                    gt = gtp.tile([128, 128], BF16, tag=f"gt{mc}", name=f"gt{mc}_{b}")
                    nc.vector.tensor_copy(gt[:], tp[:])
                    gts.append(gt)

                # ---- scores/exp interleaved with attention at round granularity ----
                # PE executes its stream in order, so alternate 4 score MMs
                # (one round) with 8 attention-accumulation MMs; ScalarE exps
                # overlap the attention matmuls.
                ets = []
                for mc in range(MC):
                    et = expp.tile([128, N], BF16, tag=f"expT{mc}", name=f"expT{mc}_{b}")
                    ets.append(et)

                # deferred attention work-units, 8 accumulation MMs each:
                # (chunk, 'attn') and (chunk, 'den') + epilogue after 'den'
                aps_map = {}

                def unit_attn(i):
                    nsl = slice(i * NCH, (i + 1) * NCH)
                    aps = psa.tile([128, NCH], F32, tag="a", name=f"aps{b}_{i}")
                    aps_map[i] = aps
                    for mc in range(MC):
                        nc.tensor.matmul(aps[:], gts[mc][:], ets[mc][:, nsl],
                                         start=(mc == 0), stop=(mc == MC - 1),
                                         skip_group_check=True)

                def unit_den_epi(i):
                    nsl = slice(i * NCH, (i + 1) * NCH)
                    aps = aps_map.pop(i)
                    dps = psd.tile([128, NCH], F32, tag="d", name=f"dps{b}_{i}")
                    xr = xrp.tile([128, 1024], F32, tag="xr", name=f"xr{b}_{i}")
                    for oc in range(2):
                        nc.sync.dma_start(xr[:, oc * NCH:(oc + 1) * NCH],
                                          x_d[b, oc * 128:(oc + 1) * 128, nsl])
                    for mc in range(MC):
                        nc.tensor.matmul(dps[:], ones[:], ets[mc][:, nsl],
                                         start=(mc == 0), stop=(mc == MC - 1),
                                         skip_group_check=True)
                    rec = smallp.tile([128, NCH], F32, tag="rec", name=f"rec{b}_{i}")
                    nc.vector.reciprocal_approx_fast(rec[:], dps[:])
                    at = smallp.tile([128, NCH], BF16, tag="attn", name=f"at{b}_{i}")
                    nc.vector.scalar_tensor_tensor(
                        at[:], aps[:], 1.0, rec[:],
                        mybir.AluOpType.bypass, mybir.AluOpType.mult)
                    op0 = psa.tile([128, NCH], F32, tag="a", name=f"op0_{b}_{i}")
                    nc.tensor.matmul(op0[:], wo[:, 0:128], at[:], start=True, stop=True)
                    op1 = psd.tile([128, NCH], F32, tag="d", name=f"op1_{b}_{i}")
                    nc.tensor.matmul(op1[:], wo[:, 128:256], at[:], start=True, stop=True)
                    for oc, ops in ((0, op0), (1, op1)):
                        osb = outp.tile([128, NCH], F32, tag="osb",
                                        name=f"osb{b}_{i}_{oc}")
                        nc.vector.scalar_tensor_tensor(
                            osb[:], ops[:], 1.0, xr[:, oc * NCH:(oc + 1) * NCH],
                            mybir.AluOpType.bypass, mybir.AluOpType.add)
                        nc.sync.dma_start(out_d[b, oc * 128:(oc + 1) * 128, nsl],
                                          osb[:])

                units = []
                for i in range(NNCH):
                    units.append(lambda i=i: unit_attn(i))
                    units.append(lambda i=i: unit_den_epi(i))
                uidx = 0

                for qt in range(5):
                    if qt < 4:
                        qsl = slice(qt * 1024, (qt + 1) * 1024)
                        for r in range(4):
                            mc_a, mc_b = 2 * r, 2 * r + 1
                            spa = psb.tile([128, 1024], F32, tag="big",
                                           name=f"spa{b}_{qt}_{r}")
                            spb = psb.tile([128, 1024], F32, tag="big",
                                           name=f"spb{b}_{qt}_{r}")
                            for hf in range(2):
                                nsl = slice(qt * 1024 + hf * 512, qt * 1024 + (hf + 1) * 512)
                                osl = slice(hf * 512, (hf + 1) * 512)
                                nc.tensor.matmul(
                                    spa[:, osl], ph2[0:32, mc_a * 128:(mc_a + 1) * 128],
                                    th2[0:32, nsl], start=True, stop=True)
                                nc.tensor.matmul(
                                    spb[:, osl], ph2[32:64, mc_b * 128:(mc_b + 1) * 128],
                                    th2[32:64, nsl], start=True, stop=True)
                            nc.scalar.activation(ets[mc_a][:, qsl], spa[:],
                                                 mybir.ActivationFunctionType.Exp)
                            nc.scalar.activation(ets[mc_b][:, qsl], spb[:],
                                                 mybir.ActivationFunctionType.Exp)
                            if qt >= 1 and uidx < len(units):
                                units[uidx](); uidx += 1
                    else:
                        while uidx < len(units):
                            units[uidx](); uidx += 1

    nc.compile()
    return nc


_NC_CACHE = None


def _get_nc():
    global _NC_CACHE
    if _NC_CACHE is None:
        _NC_CACHE = build_kernel()
    return _NC_CACHE


def prep_inputs(x, w_theta, w_phi, w_g, w_o, gamma):
    """Host-side prep: shard x over 8 cores; transpose/scale/pack weights."""
    x = np.asarray(x, dtype=np.float32).reshape(B, C, N)
    w_theta = np.asarray(w_theta, dtype=np.float32)
    w_phi = np.asarray(w_phi, dtype=np.float32)
    w_g = np.asarray(w_g, dtype=np.float32)
    w_o = np.asarray(w_o, dtype=np.float32)
    gamma = np.float32(gamma)

    # combined projection weight: [th th ph ph] along output dim
    wqT = np.concatenate([w_theta.T, w_theta.T, w_phi.T, w_phi.T], axis=1)  # [256,128]
    wq = np.ascontiguousarray(wqT.reshape(2, 128, 128))
    wgq = np.ascontiguousarray(w_g.T.reshape(2, 128, C2))
    wo = np.ascontiguousarray((gamma * w_o).T)
    ident = np.eye(128, dtype=np.float32)

    in_maps = []
    for core in range(NCORES):
        shard = np.ascontiguousarray(x[core * BPC:(core + 1) * BPC])
        in_maps.append({"x": shard, "wq": wq, "wg": wgq, "wo": wo, "ident": ident})
    return in_maps


def run(inputs, trace=False, **kw):
    nc = _get_nc()
    in_maps = prep_inputs(**inputs)
    res = run_bass_kernel_spmd(nc, in_maps, core_ids=list(range(NCORES)),
                               trace=trace, **kw)
    outs = [res.results[i]["out"] for i in range(NCORES)]
    full = np.concatenate(outs, axis=0).reshape(B, C, H, W).astype(np.float32)
    return full, res


def kernel(**inputs):
    full, _ = run(inputs, trace=False)
    return full
